# revision 1
# baseline (speedup 1.0000x reference)
"""Trainium2 Bass kernel for nn_CosSimConv2D.

Math (per sample b):
  s    = im2col3x3(x) @ w_hat           where w_hat = w / (||w||_col + qv)
  out  = sign(s) * exp(a_u/2 * (ln(s^2) - ln(box)))
  box  = 3x3 box-filter of per-pixel sum(x^2)  (= ||im2col row||^2)
  a    = softmax(p)
(The eps=1e-12 terms of the reference are dropped; they are ~1e-7-relative.)

GEMM precision: x and w_hat are each split hi+lo in bf16; three product
terms (xh@wh + xh@wl + xl@wh) recover ~fp32-grade dot products.
Data-parallel over batch: core b computes sample b.

Layouts on device (per core):
  alloc1 (128p, 130*130) bf16 : partitions 0-63 = x_hi^T padded image,
                                partitions 64-127 = x_lo^T padded image
  alloc2 (128p, 130*130) bf16 : partitions 0-63 = x_hi^T,
                                partitions 64-127 = x_hi^T shifted +1 col
  out tiles: (128 units, 512 pixels) in PSUM -> epilogue -> DRAM (128u, 16384pix)
Host transposes the per-core result back to (H, W, UNITS).
"""

import sys

sys.path.insert(0, "/opt/trn_rl_repo")

import numpy as np
import ml_dtypes

import concourse.bass as bass
import concourse.mybir as mybir
import concourse.tile as tile
from concourse import bacc
from concourse.bass_utils import run_bass_kernel_spmd
from concourse.masks import make_identity

BF16 = mybir.dt.bfloat16
F32 = mybir.dt.float32
AF = mybir.ActivationFunctionType

B, H, W, C, UNITS = 8, 128, 128, 64, 128
HW = H * W  # 16384
HP, WP = H + 2, W + 2  # 130x130 padded image
NTAP = 9
SLAB = 16  # image rows per streaming slab
NSLAB = H // SLAB
TILE_ROWS = 4  # image rows per output tile -> N = 512
NT = H // TILE_ROWS  # 32 output tiles
NPIX = TILE_ROWS * W  # 512

_CACHE = {}


def _build():
    nc = bacc.Bacc("TRN2", target_bir_lowering=False, debug=False)

    x_d = nc.dram_tensor("x", [HW, C], F32, kind="ExternalInput")
    wt13_d = nc.dram_tensor("wt13", [NTAP, 128, UNITS], BF16, kind="ExternalInput")
    wt2p_d = nc.dram_tensor("wt2p", [3, 128, UNITS], BF16, kind="ExternalInput")
    ws_last_d = nc.dram_tensor("ws_last", [3, 64, UNITS], BF16, kind="ExternalInput")
    a2_d = nc.dram_tensor("a2", [128, 1], F32, kind="ExternalInput")
    band_d = nc.dram_tensor("band", [128, 128], BF16, kind="ExternalInput")
    out_d = nc.dram_tensor("out", [128, HW], F32, kind="ExternalOutput")

    with tile.TileContext(nc) as tc:
        with (
            tc.tile_pool(name="const", bufs=1) as constp,
            tc.tile_pool(name="big", bufs=1) as bigp,
            tc.tile_pool(name="slab", bufs=2) as slabp,
            tc.tile_pool(name="epi", bufs=3) as epip,
            tc.tile_pool(name="ptr", bufs=2, space="PSUM") as ptrp,
            tc.tile_pool(name="pmm", bufs=2, space="PSUM") as pmmp,
            tc.tile_pool(name="pmisc", bufs=1, space="PSUM") as pmiscp,
        ):
            # ---- constants ----
            wt13 = constp.tile([128, NTAP, UNITS], BF16, tag="wt13")
            nc.sync.dma_start(out=wt13, in_=wt13_d.ap().rearrange("t k u -> k t u"))
            wt2p = constp.tile([128, 3, UNITS], BF16, tag="wt2p")
            nc.sync.dma_start(out=wt2p, in_=wt2p_d.ap().rearrange("t k u -> k t u"))
            ws_last = constp.tile([64, 3, UNITS], BF16, tag="wsl")
            nc.sync.dma_start(out=ws_last, in_=ws_last_d.ap().rearrange("t k u -> k t u"))
            a2 = constp.tile([128, 1], F32, tag="a2")
            nc.sync.dma_start(out=a2, in_=a2_d[:, :])
            band = constp.tile([128, 128], BF16, tag="band")
            nc.sync.dma_start(out=band, in_=band_d[:, :])
            ident = constp.tile([128, 128], BF16, tag="ident")
            make_identity(nc, ident)

            # ---- big persistent buffers ----
            alloc1 = bigp.tile([128, HP * WP], BF16, tag="alloc1")
            alloc2 = bigp.tile([128, HP * WP], BF16, tag="alloc2")
            a1v = alloc1.rearrange("p (hp wp) -> p hp wp", wp=WP)
            a2v = alloc2.rearrange("p (hp wp) -> p hp wp", wp=WP)
            lbc = bigp.tile([128, HW], BF16, tag="lbc")
            s2p = bigp.tile([128, HP], BF16, tag="s2p")  # (w, padded h)

            # zero borders of alloc1/alloc2: rows hp=0,129 and cols wp=0,129
            for av in (a1v, a2v):
                nc.vector.memset(av[:, 0, :], 0.0)
                nc.vector.memset(av[:, HP - 1, :], 0.0)
                nc.vector.memset(av[:, :, 0], 0.0)
                nc.vector.memset(av[:, :, WP - 1], 0.0)
            nc.vector.memset(s2p[:, 0:1], 0.0)
            nc.vector.memset(s2p[:, HP - 1 : HP], 0.0)

            xv = x_d.ap().rearrange("(h w) c -> h w c", w=W)

            # ---- pass 1: sum-of-squares image for norms (all slabs) ----
            for s in range(NSLAB):
                h0 = s * SLAB
                xnat = slabp.tile([128, SLAB, C], F32, tag="xnata")
                nc.sync.dma_start(
                    out=xnat, in_=xv[h0 : h0 + SLAB].rearrange("h w c -> w h c")
                )
                xsq = slabp.tile([128, SLAB, C], BF16, tag="xsq")
                nc.scalar.activation(out=xsq, in_=xnat, func=AF.Square)
                with nc.allow_low_precision(reason="s2 bf16 ~5e-4 rel; out err ~a*2.5e-4"):
                    nc.vector.tensor_reduce(
                        out=s2p[:, 1 + h0 : 1 + h0 + SLAB],
                        in_=xsq,
                        axis=mybir.AxisListType.X,
                        op=mybir.AluOpType.add,
                    )

            # ---- norm image: box filter + log + broadcast ----
            timg = bigp.tile([128, 128], BF16, tag="timg")  # (w, h) h-boxed
            nc.vector.tensor_tensor(
                out=timg, in0=s2p[:, 0:128], in1=s2p[:, 1:129], op=mybir.AluOpType.add
            )
            nc.vector.tensor_tensor(
                out=timg, in0=timg, in1=s2p[:, 2:130], op=mybir.AluOpType.add
            )
            boxp = pmiscp.tile([128, 128], F32, tag="boxp")
            nc.tensor.matmul(boxp, band, timg, start=True, stop=True)
            lpos = bigp.tile([128, 128], BF16, tag="lpos")
            nc.scalar.activation(out=lpos, in_=boxp, func=AF.Ln)
            lneg = bigp.tile([128, 128], BF16, tag="lneg")
            nc.vector.tensor_scalar_mul(out=lneg, in0=lpos, scalar1=-1.0)
            ltp = pmiscp.tile([128, 128], BF16, tag="ltp")
            nc.tensor.transpose(ltp, lneg, ident)
            lrow = bigp.tile([128, 128], BF16, tag="lrow")  # (h, w)
            nc.vector.tensor_copy(out=lrow, in_=ltp)
            nc.sync.dma_start(out=lbc[0:1, :], in_=lrow)
            n = 1
            while n < 128:
                nc.sync.dma_start(out=lbc[n : 2 * n, :], in_=lbc[0:n, :])
                n *= 2

            # ---- pass 2: transposes interleaved with GEMM tiles ----
            def slab_prep(s):
                h0 = s * SLAB
                xnat = slabp.tile([128, SLAB, C], F32, tag="xnatb")
                nc.sync.dma_start(
                    out=xnat, in_=xv[h0 : h0 + SLAB].rearrange("h w c -> w h c")
                )
                packed = slabp.tile([128, SLAB, 2, C], BF16, tag="packed")
                nc.vector.tensor_copy(out=packed[:, :, 0, :], in_=xnat)
                nc.vector.tensor_tensor(
                    out=packed[:, :, 1, :],
                    in0=xnat,
                    in1=packed[:, :, 0, :],
                    op=mybir.AluOpType.subtract,
                )
                for g in range(SLAB // 8):
                    ptr = ptrp.tile([128, 8, 128], BF16, tag="ptr")
                    for r in range(8):
                        hl = g * 8 + r
                        nc.tensor.transpose(
                            ptr[:, r, :],
                            packed[:, hl, :, :].rearrange("p t c -> p (t c)"),
                            ident,
                        )
                    hp0 = h0 + g * 8 + 1
                    nc.vector.tensor_copy(out=a1v[:, hp0 : hp0 + 8, 1 : 1 + W], in_=ptr)
                    nc.sync.dma_start(
                        out=a2v[0:64, hp0 : hp0 + 8, :],
                        in_=a1v[0:64, hp0 : hp0 + 8, :],
                    )
                    nc.sync.dma_start(
                        out=alloc2.rearrange("p (hp wp) -> p hp wp", wp=WP)[
                            64:128, hp0 : hp0 + 8, 0 : WP - 1
                        ],
                        in_=alloc1.rearrange("p (hp wp) -> p hp wp", wp=WP)[
                            0:64, hp0 : hp0 + 8, 1:WP
                        ],
                    )

            # ---- GEMM + epilogue per output tile ----
            def emit_tile(j):
                hh = j * TILE_ROWS
                ps = pmmp.tile([128, TILE_ROWS, W], F32, tag="ps")
                first = True
                for ty in range(3):
                    for tx in range(3):
                        nc.tensor.matmul(
                            ps,
                            wt13[:, ty * 3 + tx, :],
                            a1v[:, hh + ty : hh + ty + TILE_ROWS, tx : tx + W],
                            start=first,
                            stop=False,
                        )
                        first = False
                for ty in range(3):
                    nc.tensor.matmul(
                        ps,
                        wt2p[:, ty, :],
                        a2v[:, hh + ty : hh + ty + TILE_ROWS, 0:W],
                        start=False,
                        stop=False,
                    )
                for ty in range(3):
                    nc.tensor.matmul(
                        ps,
                        ws_last[:, ty, :],
                        a2v[0:64, hh + ty : hh + ty + TILE_ROWS, 2 : 2 + W],
                        start=False,
                        stop=(ty == 2),
                    )
                psf = ps.rearrange("p r w -> p (r w)")
                sq = epip.tile([128, NPIX], BF16, tag="sq")
                nc.scalar.activation(out=sq, in_=psf, func=AF.Square)
                sgn = epip.tile([128, NPIX], F32, tag="sgn")
                nc.scalar.activation(out=sgn, in_=psf, func=AF.Sign)
                v = epip.tile([128, NPIX], BF16, tag="v")
                nc.scalar.activation(out=v, in_=sq, func=AF.Ln)
                v2 = epip.tile([128, NPIX], BF16, tag="v2")
                nc.vector.tensor_tensor(
                    out=v2,
                    in0=v,
                    in1=lbc[:, j * NPIX : (j + 1) * NPIX],
                    op=mybir.AluOpType.add,
                )
                t3 = epip.tile([128, NPIX], F32, tag="t3")
                nc.scalar.activation(out=t3, in_=v2, func=AF.Exp, scale=a2[:, :])
                o = epip.tile([128, NPIX], F32, tag="o")
                nc.vector.tensor_tensor(
                    out=o, in0=t3, in1=sgn, op=mybir.AluOpType.mult
                )
                nc.sync.dma_start(out=out_d[:, j * NPIX : (j + 1) * NPIX], in_=o)

            emitted = 0
            for s in range(NSLAB):
                slab_prep(s)
                while emitted <= min(4 * s + 2, NT - 1):
                    emit_tile(emitted)
                    emitted += 1
            while emitted < NT:
                emit_tile(emitted)
                emitted += 1

    nc.compile()
    return nc


def _host_prep(w, p, q):
    EPS = 1e-12
    w64 = w[0].astype(np.float64)  # (576, 128)
    qv = (q.astype(np.float64) ** 2 / 10.0)[0]
    wn = np.sqrt(np.maximum((w64**2).sum(0), EPS)) + qv
    what = (w64 / wn).astype(np.float32)
    wh = what.astype(ml_dtypes.bfloat16)
    wl = (what - wh.astype(np.float32)).astype(ml_dtypes.bfloat16)

    def tap(a, k):
        return np.ascontiguousarray(a[k * 64 : (k + 1) * 64, :])

    wt13 = np.stack([np.vstack([tap(wh, k), tap(wh, k)]) for k in range(9)])
    wt2p = np.stack(
        [np.vstack([tap(wl, 3 * ty + 0), tap(wl, 3 * ty + 1)]) for ty in range(3)]
    )
    ws_last = np.stack([tap(wl, 2), tap(wl, 5), tap(wl, 8)])

    pe = np.exp(p.astype(np.float64) - p.astype(np.float64).max())
    a = pe / pe.sum()
    a2 = (a * 0.5).astype(np.float32).reshape(128, 1)

    band = np.zeros((128, 128), dtype=np.float32)
    for i in range(128):
        band[i, max(0, i - 1) : i + 2] = 1.0
    band = band.astype(ml_dtypes.bfloat16)
    return wt13, wt2p, ws_last, a2, band


LAST_RESULTS = None


def kernel(inputs, w, p, q):
    global LAST_RESULTS
    if "nc" not in _CACHE:
        _CACHE["nc"] = _build()
    nc = _CACHE["nc"]

    wt13, wt2p, ws_last, a2, band = _host_prep(w, p, q)
    xs = np.ascontiguousarray(inputs.reshape(B, HW, C).astype(np.float32))
    in_maps = [
        {
            "x": xs[b],
            "wt13": wt13,
            "wt2p": wt2p,
            "ws_last": ws_last,
            "a2": a2,
            "band": band,
        }
        for b in range(B)
    ]
    import os

    trace = bool(int(os.environ.get("KERNEL_TRACE", "0")))
    res = run_bass_kernel_spmd(nc, in_maps, core_ids=list(range(B)), trace=trace)
    LAST_RESULTS = res
    out = np.stack(
        [res.results[b]["out"].T.reshape(H, W, UNITS) for b in range(B)]
    ).astype(np.float32)
    return out



# revision 2
# speedup vs baseline: 1.3454x; 1.3454x over previous
"""Trainium2 Bass kernel for nn_CosSimConv2D.

Math (per sample b):
  s    = im2col3x3(x) @ w_hat           where w_hat = w / (||w||_col + qv)
  out  = sign(s) * exp(a_u/2 * (ln(s^2) - ln(box)))
  box  = 3x3 box-filter of per-pixel sum(x^2)  (= ||im2col row||^2)
  a    = softmax(p)
(The eps=1e-12 terms of the reference are dropped; they are ~1e-7-relative.)

GEMM precision: x and w_hat are each split hi+lo in bf16; three product
terms (xh@wh + xh@wl + xl@wh) recover ~fp32-grade dot products.
Data-parallel over batch: core b computes sample b.

Layouts on device (per core):
  alloc1 (128p, 130*130) bf16 : partitions 0-63 = x_hi^T padded image,
                                partitions 64-127 = x_lo^T padded image
  alloc2 (128p, 130*130) bf16 : partitions 0-63 = x_hi^T,
                                partitions 64-127 = x_hi^T shifted +1 col
  out tiles: (128 units, 512 pixels) in PSUM -> epilogue -> DRAM (128u, 16384pix)
Host transposes the per-core result back to (H, W, UNITS).
"""

import sys

sys.path.insert(0, "/opt/trn_rl_repo")

import numpy as np
import ml_dtypes

import concourse.bass as bass
import concourse.mybir as mybir
import concourse.tile as tile
from concourse import bacc
from concourse.bass_utils import run_bass_kernel_spmd
from concourse.masks import make_identity

BF16 = mybir.dt.bfloat16
F32 = mybir.dt.float32
AF = mybir.ActivationFunctionType

B, H, W, C, UNITS = 8, 128, 128, 64, 128
HW = H * W  # 16384
HP, WP = H + 2, W + 2  # 130x130 padded image
NTAP = 9
SLAB = 16  # image rows per streaming slab
NSLAB = H // SLAB
TILE_ROWS = 4  # image rows per output tile -> N = 512
NT = H // TILE_ROWS  # 32 output tiles
NPIX = TILE_ROWS * W  # 512

_CACHE = {}


def _build():
    nc = bacc.Bacc("TRN2", target_bir_lowering=False, debug=False)

    x_d = nc.dram_tensor("x", [HW, C], F32, kind="ExternalInput")
    wt13_d = nc.dram_tensor("wt13", [NTAP, 128, UNITS], BF16, kind="ExternalInput")
    wt2p_d = nc.dram_tensor("wt2p", [3, 128, UNITS], BF16, kind="ExternalInput")
    ws_last_d = nc.dram_tensor("ws_last", [3, 64, UNITS], BF16, kind="ExternalInput")
    a2_d = nc.dram_tensor("a2", [128, 1], F32, kind="ExternalInput")
    band_d = nc.dram_tensor("band", [128, 128], BF16, kind="ExternalInput")
    out_d = nc.dram_tensor("out", [128, HW], F32, kind="ExternalOutput")

    with tile.TileContext(nc) as tc:
        with (
            tc.tile_pool(name="const", bufs=1) as constp,
            tc.tile_pool(name="big", bufs=1) as bigp,
            tc.tile_pool(name="slab", bufs=2) as slabp,
            tc.tile_pool(name="epi", bufs=3) as epip,
            tc.tile_pool(name="ptr", bufs=2, space="PSUM") as ptrp,
            tc.tile_pool(name="pmm", bufs=2, space="PSUM") as pmmp,
            tc.tile_pool(name="pmisc", bufs=1, space="PSUM") as pmiscp,
        ):
            # ---- constants ----
            wt13 = constp.tile([128, NTAP, UNITS], BF16, tag="wt13")
            nc.sync.dma_start(out=wt13, in_=wt13_d.ap().rearrange("t k u -> k t u"))
            wt2p = constp.tile([128, 3, UNITS], BF16, tag="wt2p")
            nc.sync.dma_start(out=wt2p, in_=wt2p_d.ap().rearrange("t k u -> k t u"))
            ws_last = constp.tile([64, 3, UNITS], BF16, tag="wsl")
            nc.sync.dma_start(out=ws_last, in_=ws_last_d.ap().rearrange("t k u -> k t u"))
            a2 = constp.tile([128, 1], F32, tag="a2")
            nc.sync.dma_start(out=a2, in_=a2_d[:, :])
            band = constp.tile([128, 128], BF16, tag="band")
            nc.sync.dma_start(out=band, in_=band_d[:, :])
            ident = constp.tile([128, 128], BF16, tag="ident")
            make_identity(nc, ident)

            # Preload act table set 6 (natural_log_exp_and_others), which
            # contains Square, Sign, Ln, and Exp. The auto-placement pass
            # would otherwise thrash between natural_log and exp_and_others
            # (~2.7us per reload) on every tile's Ln->Exp pair.
            nc.scalar.add_instruction(
                mybir.InstLoadActFuncSet(
                    name=nc.get_next_instruction_name(),
                    act_func_set_id=6,
                    ins=[],
                    outs=[],
                )
            )

            # ---- big persistent buffers ----
            alloc1 = bigp.tile([128, HP * WP], BF16, tag="alloc1")
            alloc2 = bigp.tile([128, HP * WP], BF16, tag="alloc2")
            a1v = alloc1.rearrange("p (hp wp) -> p hp wp", wp=WP)
            a2v = alloc2.rearrange("p (hp wp) -> p hp wp", wp=WP)
            lbc = bigp.tile([128, HW], BF16, tag="lbc")
            s2p = bigp.tile([128, HP], BF16, tag="s2p")  # (w, padded h)

            # zero borders of alloc1/alloc2: rows hp=0,129 and cols wp=0,129
            for av in (a1v, a2v):
                nc.vector.memset(av[:, 0, :], 0.0)
                nc.vector.memset(av[:, HP - 1, :], 0.0)
                nc.vector.memset(av[:, :, 0], 0.0)
                nc.vector.memset(av[:, :, WP - 1], 0.0)
            nc.vector.memset(s2p[:, 0:1], 0.0)
            nc.vector.memset(s2p[:, HP - 1 : HP], 0.0)

            xv = x_d.ap().rearrange("(h w) c -> h w c", w=W)

            # ---- pass 1: sum-of-squares image for norms (all slabs) ----
            for s in range(NSLAB):
                h0 = s * SLAB
                xnat = slabp.tile([128, SLAB, C], F32, tag="xnata")
                nc.sync.dma_start(
                    out=xnat, in_=xv[h0 : h0 + SLAB].rearrange("h w c -> w h c")
                )
                xsq = slabp.tile([128, SLAB, C], BF16, tag="xsq")
                nc.scalar.activation(out=xsq, in_=xnat, func=AF.Square)
                with nc.allow_low_precision(reason="s2 bf16 ~5e-4 rel; out err ~a*2.5e-4"):
                    nc.vector.tensor_reduce(
                        out=s2p[:, 1 + h0 : 1 + h0 + SLAB],
                        in_=xsq,
                        axis=mybir.AxisListType.X,
                        op=mybir.AluOpType.add,
                    )

            # ---- norm image: box filter + log + broadcast ----
            timg = bigp.tile([128, 128], BF16, tag="timg")  # (w, h) h-boxed
            nc.vector.tensor_tensor(
                out=timg, in0=s2p[:, 0:128], in1=s2p[:, 1:129], op=mybir.AluOpType.add
            )
            nc.vector.tensor_tensor(
                out=timg, in0=timg, in1=s2p[:, 2:130], op=mybir.AluOpType.add
            )
            boxp = pmiscp.tile([128, 128], F32, tag="boxp")
            nc.tensor.matmul(boxp, band, timg, start=True, stop=True)
            lpos = bigp.tile([128, 128], BF16, tag="lpos")
            nc.scalar.activation(out=lpos, in_=boxp, func=AF.Ln)
            lneg = bigp.tile([128, 128], BF16, tag="lneg")
            nc.vector.tensor_scalar_mul(out=lneg, in0=lpos, scalar1=-1.0)
            ltp = pmiscp.tile([128, 128], BF16, tag="ltp")
            nc.tensor.transpose(ltp, lneg, ident)
            lrow = bigp.tile([128, 128], BF16, tag="lrow")  # (h, w)
            nc.vector.tensor_copy(out=lrow, in_=ltp)
            nc.sync.dma_start(out=lbc[0:1, :], in_=lrow)
            n = 1
            while n < 128:
                nc.sync.dma_start(out=lbc[n : 2 * n, :], in_=lbc[0:n, :])
                n *= 2

            # ---- pass 2: transposes interleaved with GEMM tiles ----
            def slab_prep(s):
                h0 = s * SLAB
                xnat = slabp.tile([128, SLAB, C], F32, tag="xnatb")
                nc.sync.dma_start(
                    out=xnat, in_=xv[h0 : h0 + SLAB].rearrange("h w c -> w h c")
                )
                packed = slabp.tile([128, SLAB, 2, C], BF16, tag="packed")
                nc.vector.tensor_copy(out=packed[:, :, 0, :], in_=xnat)
                nc.vector.tensor_tensor(
                    out=packed[:, :, 1, :],
                    in0=xnat,
                    in1=packed[:, :, 0, :],
                    op=mybir.AluOpType.subtract,
                )
                for g in range(SLAB // 8):
                    ptr = ptrp.tile([128, 8, 128], BF16, tag="ptr")
                    for r in range(8):
                        hl = g * 8 + r
                        nc.tensor.transpose(
                            ptr[:, r, :],
                            packed[:, hl, :, :].rearrange("p t c -> p (t c)"),
                            ident,
                        )
                    hp0 = h0 + g * 8 + 1
                    nc.vector.tensor_copy(out=a1v[:, hp0 : hp0 + 8, 1 : 1 + W], in_=ptr)
                    nc.sync.dma_start(
                        out=a2v[0:64, hp0 : hp0 + 8, :],
                        in_=a1v[0:64, hp0 : hp0 + 8, :],
                    )
                    nc.sync.dma_start(
                        out=alloc2.rearrange("p (hp wp) -> p hp wp", wp=WP)[
                            64:128, hp0 : hp0 + 8, 0 : WP - 1
                        ],
                        in_=alloc1.rearrange("p (hp wp) -> p hp wp", wp=WP)[
                            0:64, hp0 : hp0 + 8, 1:WP
                        ],
                    )

            # ---- GEMM + epilogue per output tile ----
            def emit_tile(j):
                hh = j * TILE_ROWS
                ps = pmmp.tile([128, TILE_ROWS, W], F32, tag="ps")
                first = True
                for ty in range(3):
                    for tx in range(3):
                        nc.tensor.matmul(
                            ps,
                            wt13[:, ty * 3 + tx, :],
                            a1v[:, hh + ty : hh + ty + TILE_ROWS, tx : tx + W],
                            start=first,
                            stop=False,
                        )
                        first = False
                for ty in range(3):
                    nc.tensor.matmul(
                        ps,
                        wt2p[:, ty, :],
                        a2v[:, hh + ty : hh + ty + TILE_ROWS, 0:W],
                        start=False,
                        stop=False,
                    )
                for ty in range(3):
                    nc.tensor.matmul(
                        ps,
                        ws_last[:, ty, :],
                        a2v[0:64, hh + ty : hh + ty + TILE_ROWS, 2 : 2 + W],
                        start=False,
                        stop=(ty == 2),
                    )
                psf = ps.rearrange("p r w -> p (r w)")
                sq = epip.tile([128, NPIX], BF16, tag="sq")
                nc.scalar.activation(out=sq, in_=psf, func=AF.Square)
                sgn = epip.tile([128, NPIX], F32, tag="sgn")
                nc.scalar.activation(out=sgn, in_=psf, func=AF.Sign)
                v = epip.tile([128, NPIX], BF16, tag="v")
                nc.scalar.activation(out=v, in_=sq, func=AF.Ln)
                v2 = epip.tile([128, NPIX], BF16, tag="v2")
                nc.vector.tensor_tensor(
                    out=v2,
                    in0=v,
                    in1=lbc[:, j * NPIX : (j + 1) * NPIX],
                    op=mybir.AluOpType.add,
                )
                t3 = epip.tile([128, NPIX], F32, tag="t3")
                nc.scalar.activation(out=t3, in_=v2, func=AF.Exp, scale=a2[:, :])
                o = epip.tile([128, NPIX], F32, tag="o")
                nc.vector.tensor_tensor(
                    out=o, in0=t3, in1=sgn, op=mybir.AluOpType.mult
                )
                nc.sync.dma_start(out=out_d[:, j * NPIX : (j + 1) * NPIX], in_=o)

            emitted = 0
            for s in range(NSLAB):
                slab_prep(s)
                while emitted <= min(4 * s + 2, NT - 1):
                    emit_tile(emitted)
                    emitted += 1
            while emitted < NT:
                emit_tile(emitted)
                emitted += 1

    nc.compile()
    return nc


def _host_prep(w, p, q):
    EPS = 1e-12
    w64 = w[0].astype(np.float64)  # (576, 128)
    qv = (q.astype(np.float64) ** 2 / 10.0)[0]
    wn = np.sqrt(np.maximum((w64**2).sum(0), EPS)) + qv
    what = (w64 / wn).astype(np.float32)
    wh = what.astype(ml_dtypes.bfloat16)
    wl = (what - wh.astype(np.float32)).astype(ml_dtypes.bfloat16)

    def tap(a, k):
        return np.ascontiguousarray(a[k * 64 : (k + 1) * 64, :])

    wt13 = np.stack([np.vstack([tap(wh, k), tap(wh, k)]) for k in range(9)])
    wt2p = np.stack(
        [np.vstack([tap(wl, 3 * ty + 0), tap(wl, 3 * ty + 1)]) for ty in range(3)]
    )
    ws_last = np.stack([tap(wl, 2), tap(wl, 5), tap(wl, 8)])

    pe = np.exp(p.astype(np.float64) - p.astype(np.float64).max())
    a = pe / pe.sum()
    a2 = (a * 0.5).astype(np.float32).reshape(128, 1)

    band = np.zeros((128, 128), dtype=np.float32)
    for i in range(128):
        band[i, max(0, i - 1) : i + 2] = 1.0
    band = band.astype(ml_dtypes.bfloat16)
    return wt13, wt2p, ws_last, a2, band


LAST_RESULTS = None


def kernel(inputs, w, p, q):
    global LAST_RESULTS
    if "nc" not in _CACHE:
        _CACHE["nc"] = _build()
    nc = _CACHE["nc"]

    wt13, wt2p, ws_last, a2, band = _host_prep(w, p, q)
    xs = np.ascontiguousarray(inputs.reshape(B, HW, C).astype(np.float32))
    in_maps = [
        {
            "x": xs[b],
            "wt13": wt13,
            "wt2p": wt2p,
            "ws_last": ws_last,
            "a2": a2,
            "band": band,
        }
        for b in range(B)
    ]
    import os

    trace = bool(int(os.environ.get("KERNEL_TRACE", "0")))
    res = run_bass_kernel_spmd(nc, in_maps, core_ids=list(range(B)), trace=trace)
    LAST_RESULTS = res
    out = np.stack(
        [res.results[b]["out"].T.reshape(H, W, UNITS) for b in range(B)]
    ).astype(np.float32)
    return out

